# revision 20
# baseline (speedup 1.0000x reference)
"""Trainium2 Bass/Tile kernel for BasicCondConvBlock (E=1):
two CondConv1d(k=3,pad=1)+BN(eval)+LeakyReLU(0.1) blocks + MaxPool1d(2).

With a single expert, CondConv reduces to y_i = r_i * (conv(x_i, W) + b)
with a shared weight; routing r_i + conv bias + BatchNorm fold into one
per-(sample,channel) affine applied at PSUM-drain time:
    out = LeakyReLU( (r_i*s_c) * z + (r_i*b_c*s_c + be_c - rm_c*s_c) )

Everything off the fp32 PSUM accumulate runs in bf16 (tolerance 2e-2;
measured ~5e-3): conv matmuls (1 col/cycle vs ~2 for fp32r, weights get
FastWeightLoad), routing sums/matmuls (2-byte DVE ops hit 2x mode), and the
final output (halves the out-DMA bytes; the host converts back to f32).
Block-1 packs taps 0+1 into one K=128 matmul: x is host-duplicated into
partitions 64..127 shifted by one column, so each 512-col chunk is 2
matmuls (taps01 K=128 + tap2 K=64) instead of 3.

PE-queue discipline (the engine queue is in-order, so anything emitted
before conv matmuls stalls them): a zero-tile warm-up burst at t=0 covers
the x0-DMA latency and warms the HAM clock gate to full speed (a cold PE
runs at 1.2 GHz; one >3us idle gap mid-kernel re-throttles it, so the x
DMAs are spread over the SWDGE + sync + scalar queues to keep the conv
stream fed).  Each routing matmul is emitted AFTER the conv matmuls that
hide its DVE-reduce latency; block-2 of sample s is interleaved right
after block-1 of sample s+1 so out-DMAs spread across the whole kernel
instead of piling into a tail.

The per-sample x row-sum is two half-width DVE reduces (top partitions sum
the first half of x, the shifted bottom copy sums the second half); the
routing matmul's replicated fc weight spans all 128 partitions so the
contraction adds the two partials for free.

Drains: block-1 writes y1 as bf16 via ScalarE Prelu(affine) per [128,1024]
PSUM tile with fused row-sum (feeds block-2 routing); one tile drains on
VectorE.  Block-2 drains 10-of-16 tiles as (a) VectorE 3D-AP max-pool
PSUM->bf16 then half-width ScalarE Prelu(affine) (exact: scale>0 keeps
affine+Prelu monotone) and 6-of-16 as (b) full-width ScalarE
Prelu(affine)->bf16 then VectorE strided pool, splitting PSUM reads across
engines.  Sharding: pure data parallel over batch (32 -> 4x8).
"""

import numpy as np

N_CORES = 8
B, CIN, W = 32, 64, 2048
C1, C2 = 128, 256
BL = B // N_CORES  # samples per core
EPS = 1e-5
SLOPE = 0.1
WT = 512           # conv output tile width (one PSUM bank of fp32)
WO = W // 2        # pooled output width
W2T = 2 * WT       # PSUM tiles span two banks; drains amortize per-op overhead
NWARM = 14         # HAM warm-up matmuls; bridges the x0 DMA (no PE idle)
H2 = (W + 2) // 2  # x half-DMA split point

# wpk (bf16) column layout: [0:128] w1 taps01 lhsT (rows 0-63 tap0, 64-127
# tap1), [128:256] w1 tap2 (rows 0-63), [256:384] fcw1/W replicated over all
# rows (the rt1 matmul contracts the two half-sum partials), [384:512]
# fcw2/W replicated, [512:1280] w2 lhsT [i, k*C2+o]
T2 = 128
FC1OFF = 256
FC2OFF = 384
WAEND = 512
W2OFF = 512
NWB = W2OFF + 3 * C2
# cpk (f32) consts: s1, t11, t21, s2a, s2b, t12a, t12b, t22a, t22b, fcb1,
# fcb2, then an fp32 fcw2/W replica (the rt2 PSUM-accumulate matmuls need
# fp32 on both sides)
NCV = 11
NCC = NCV + C1

TRACE = False
LAST_RESULT = None

_built = None


def _build():
    global _built
    if _built is not None:
        return _built

    import concourse.bacc as bacc
    import concourse.mybir as mybir
    from concourse import tile
    from contextlib import ExitStack

    f32 = mybir.dt.float32
    bf16 = mybir.dt.bfloat16
    Alu = mybir.AluOpType
    Act = mybir.ActivationFunctionType
    Ax = mybir.AxisListType

    nc = bacc.Bacc("TRN2", target_bir_lowering=False, debug=False)

    xd = nc.declare_dram_parameter("xpk", [BL, C1, W + 2], bf16, isOutput=False)
    wd = nc.declare_dram_parameter("wpk", [C1, NWB], bf16, isOutput=False)
    cd = nc.declare_dram_parameter("cpk", [C1, NCC], f32, isOutput=False)
    od = nc.declare_dram_parameter("out", [BL, C2, WO], bf16, isOutput=True)
    x_ap, w_ap, c_ap, o_ap = xd.ap(), wd.ap(), cd.ap(), od.ap()

    with tile.TileContext(nc) as tc:
        with ExitStack() as ctx:
            consts = ctx.enter_context(tc.tile_pool(name="consts", bufs=1))
            xpool = ctx.enter_context(tc.tile_pool(name="xp", bufs=BL))
            y1pool = ctx.enter_context(tc.tile_pool(name="y1p", bufs=BL))
            pmp = ctx.enter_context(tc.tile_pool(name="pmp", bufs=6))
            outp = ctx.enter_context(tc.tile_pool(name="outp", bufs=8))
            small = ctx.enter_context(tc.tile_pool(name="small", bufs=1))
            psum = ctx.enter_context(tc.tile_pool(name="psum", bufs=3, space="PSUM"))
            psmall = ctx.enter_context(tc.tile_pool(name="psm", bufs=1, space="PSUM"))
            pwarm = ctx.enter_context(tc.tile_pool(name="pwm", bufs=1, space="PSUM"))

            # --- input DMAs.  SWDGE (gpsimd) is ~2x the ring rate: it
            # carries x0 first (gates the first conv), then x1's second half
            # and x3; the sync ring brings x1's first half and x2; the
            # scalar ring brings w1+routing weights, consts, then w2.
            xts = [
                xpool.tile([C1, W + 2], bf16, tag="xt", name=f"xt{s}")
                for s in range(BL)
            ]
            ws = consts.tile([C1, NWB], bf16)
            cs = consts.tile([C1, NCC], f32)
            nc.gpsimd.dma_start(out=xts[0][0:CIN], in_=x_ap[0][0:CIN])
            nc.gpsimd.dma_start(out=xts[1][0:CIN], in_=x_ap[1][0:CIN])
            nc.gpsimd.dma_start(out=xts[3][:], in_=x_ap[3])
            nc.sync.dma_start(out=xts[0][CIN:C1], in_=x_ap[0][CIN:C1])
            nc.sync.dma_start(out=xts[1][CIN:C1], in_=x_ap[1][CIN:C1])
            nc.sync.dma_start(out=xts[2][CIN:C1], in_=x_ap[2][CIN:C1])
            nc.scalar.dma_start(out=ws[:, 0:WAEND], in_=w_ap[:, 0:WAEND])
            nc.scalar.dma_start(out=cs[:], in_=c_ap[:])
            nc.scalar.dma_start(out=ws[:, W2OFF:], in_=w_ap[:, W2OFF:])
            nc.scalar.dma_start(out=xts[2][0:CIN], in_=x_ap[2][0:CIN])

            def ck(j):
                return cs[:, j : j + 1]

            # --- HAM warm-up: zero-tile matmuls keep the PE busy while the
            # x DMAs land, so the array is at full clock for the real work.
            wz = consts.tile([C1, 640], bf16)
            nc.vector.memset(wz[:], 0.0)
            zpw = pwarm.tile([C1, WT], f32, name="warm")

            def warm_mms(n):
                for _ in range(n):
                    nc.tensor.matmul(
                        zpw[:], wz[:, 0:C1], wz[:, C1 : C1 + WT],
                        start=True, stop=True,
                    )

            warm_mms(NWARM)
            # dummy activations preload both ScalarE function tables during
            # the DMA dead time (a mid-kernel ACT_TABLE_LOAD costs 1.3us on
            # the routing critical path)
            tscr = small.tile([C1, 2], f32)
            nc.scalar.activation(tscr[:, 0:1], wz[:, 0:1], Act.Sigmoid,
                                 bias=0.0, scale=1.0)
            nc.scalar.activation(tscr[:, 1:2], wz[:, 0:1], Act.Prelu,
                                 bias=0.0, scale=1.0, alpha=SLOPE)

            m1 = small.tile([C1, BL], bf16)
            rbc1 = small.tile([C1, BL], f32)
            sc1 = small.tile([C1, BL], f32)
            bi1 = small.tile([C1, BL], f32)
            s1acc = small.tile([C1, 2 * BL], f32)
            ssum = small.tile([C1, BL], bf16)
            gscr = [small.tile([C1, W // 2], f32, name=f"gscr{s}") for s in range(2)]
            rbc2 = small.tile([C1, BL], f32)
            sc2 = small.tile([C1, 2 * BL], f32)
            bi2 = small.tile([C1, 2 * BL], f32)

            y1s = [
                y1pool.tile([C1, W + 2], bf16, tag="y1", name=f"y1_{s}")
                for s in range(BL)
            ]

            def emit_b1_mms(s):
                """Conv1 matmuls (PSUM fills) + routing row-sums, sample s."""
                xt = xts[s]
                y1 = y1s[s]
                nc.vector.memset(y1[:, 0 : W + 2 : W + 1], 0.0)
                with nc.allow_low_precision("routing logit tolerates bf16"):
                    if s < 2:
                        # two DVE half-reduces (latency-critical early
                        # samples); the top read starts at the zero pad so
                        # it misses x[1023], added back by a 1-element op
                        nc.vector.reduce_sum(
                            m1[0:CIN, s : s + 1], xt[0:CIN, 0 : W // 2], axis=Ax.X
                        )
                        nc.vector.reduce_sum(
                            m1[CIN:C1, s : s + 1], xt[CIN:C1, W // 2 : W], axis=Ax.X
                        )
                        nc.vector.tensor_tensor(
                            m1[0:CIN, s : s + 1], m1[0:CIN, s : s + 1],
                            xt[0:CIN, W // 2 : W // 2 + 1], Alu.add,
                        )
                    else:
                        # GpSimd folds the bottom (shifted) copy, which
                        # covers x exactly; one short DVE reduce finishes.
                        # Keeps the big reduces off the DVE during its
                        # busiest stretch.
                        g = gscr[s - 2]
                        nc.gpsimd.tensor_tensor(
                            g[CIN:C1, :], xt[CIN:C1, 0 : W // 2],
                            xt[CIN:C1, W // 2 : W], Alu.add,
                        )
                        nc.vector.reduce_sum(
                            m1[CIN:C1, s : s + 1], g[CIN:C1, :], axis=Ax.X
                        )
                zps = []
                for d in range(2):
                    zp = psum.tile([C1, W2T], f32, tag="zp")
                    # weight-major order: consecutive matmuls share the
                    # stationary operand, halving effective LDWEIGHTS
                    for h in range(2):
                        t0 = W2T * d + WT * h
                        nc.tensor.matmul(
                            zp[:, WT * h : WT * h + WT], ws[:, 0:C1],
                            xt[:, t0 : t0 + WT], start=True, stop=False,
                        )
                    for h in range(2):
                        t0 = W2T * d + WT * h
                        nc.tensor.matmul(
                            zp[:, WT * h : WT * h + WT], ws[0:CIN, T2 : T2 + C1],
                            xt[0:CIN, t0 + 2 : t0 + 2 + WT],
                            start=False, stop=True,
                        )
                    zps.append(zp)
                return zps

            def emit_b1_drains(s, zps):
                """Block-1 PSUM drains for sample s.  Emitted after
                emit_rt1(s) so the scale/bias reads follow their writers in
                program order (the tile scheduler takes deps from it)."""
                y1 = y1s[s]
                for d in range(2):
                    acc = s1acc[:, 2 * s + d : 2 * s + d + 1]
                    dst = y1[:, 1 + W2T * d : 1 + W2T * (d + 1)]
                    if not (s == 3 and d == 1):
                        # ScalarE drain: fused Prelu(affine) + row-sum
                        nc.scalar.activation(
                            dst, zps[d][:], Act.Prelu,
                            bias=bi1[:, s : s + 1], scale=sc1[:, s : s + 1],
                            alpha=SLOPE, accum_out=acc,
                        )
                    else:
                        # one VectorE drain balances late-kernel ScalarE load
                        ytmp = pmp.tile([C1, W2T], bf16, tag="ytmp")
                        nc.vector.tensor_scalar(
                            ytmp[:], zps[d][:],
                            sc1[:, s : s + 1], bi1[:, s : s + 1],
                            Alu.mult, Alu.add,
                        )
                        nc.vector.scalar_tensor_tensor(
                            dst, ytmp[:], SLOPE, ytmp[:], Alu.mult, Alu.max,
                            accum_out=acc,
                        )

            def emit_rt1(s):
                """Routing-1 matmul + sigmoid + scale/bias cols for sample s.
                Emitted after conv matmuls that hide the m1 reduce latency."""
                lgb = psmall.tile([C1, 1], f32, tag="sm", name=f"lg1{s}")
                if s < 2:
                    nc.tensor.matmul(
                        lgb[:], ws[:, FC1OFF : FC1OFF + C1], m1[:, s : s + 1],
                        start=True, stop=True,
                    )
                else:
                    nc.tensor.matmul(
                        lgb[:], ws[CIN:C1, FC1OFF : FC1OFF + C1],
                        m1[CIN:C1, s : s + 1], start=True, stop=True,
                    )
                nc.scalar.activation(
                    rbc1[:, s : s + 1], lgb[:], Act.Sigmoid, bias=ck(9), scale=1.0
                )
                nc.vector.tensor_scalar(
                    sc1[:, s : s + 1], ck(0), rbc1[:, s : s + 1], None, Alu.mult
                )
                nc.vector.scalar_tensor_tensor(
                    bi1[:, s : s + 1], ck(1), rbc1[:, s : s + 1], ck(2),
                    Alu.mult, Alu.add,
                )

            def emit_rt2(s):
                """Routing-2 chain for sample s, gated on its block-1 drains."""
                with nc.allow_low_precision("routing logit tolerates bf16"):
                    nc.vector.reduce_sum(
                        ssum[:, s : s + 1], s1acc[:, 2 * s : 2 * (s + 1)], axis=Ax.X
                    )
                lgb2 = psmall.tile([C1, 1], f32, tag="sm", name=f"lg2{s}")
                nc.tensor.matmul(
                    lgb2[:], ws[:, FC2OFF : FC2OFF + C1], ssum[:, s : s + 1],
                    start=True, stop=True,
                )
                nc.scalar.activation(
                    rbc2[:, s : s + 1], lgb2[:], Act.Sigmoid, bias=ck(10), scale=1.0
                )
                # both c-chunks in one paired op (consts are adjacent cols)
                nc.vector.tensor_scalar(
                    sc2[:, 2 * s : 2 * s + 2], cs[:, 3:5],
                    rbc2[:, s : s + 1], None, Alu.mult,
                )
                nc.vector.scalar_tensor_tensor(
                    bi2[:, 2 * s : 2 * s + 2], cs[:, 5:7],
                    rbc2[:, s : s + 1], cs[:, 7:9], Alu.mult, Alu.add,
                )

            ndma = [0]

            def emit_block2(s):
                """Conv2 (24 MMs) + alternating drains + out DMAs, sample s."""
                for c in range(2):
                    ot = outp.tile([C1, WO], bf16, tag="ot")
                    sc_col = sc2[:, 2 * s + c : 2 * s + c + 1]
                    bi_col = bi2[:, 2 * s + c : 2 * s + c + 1]
                    for d in range(2):
                        zp2 = psum.tile([C1, W2T], f32, tag="zp")
                        for k in range(3):
                            for h in range(2):
                                t0 = W2T * d + WT * h
                                nc.tensor.matmul(
                                    zp2[:, WT * h : WT * h + WT],
                                    ws[:, W2OFF + k * C2 + C1 * c : W2OFF + k * C2 + C1 * c + C1],
                                    y1s[s][:, t0 + k : t0 + k + WT],
                                    start=(k == 0), stop=(k == 2),
                                )
                        if d == 0 or (s, c) in ((1, 0), (3, 0)):
                            # (a) VectorE drains PSUM: one-input 3D-AP
                            # max-pool -> bf16, then ScalarE Prelu(affine) at
                            # half width (exact: scale>0 keeps it monotone)
                            pm = pmp.tile([C1, WT], bf16, tag="pm")
                            nc.vector.tensor_reduce(
                                pm[:], zp2[:].rearrange("p (a b) -> p a b", b=2),
                                axis=Ax.X, op=Alu.max,
                            )
                            nc.scalar.activation(
                                ot[:, WT * d : WT * (d + 1)], pm[:], Act.Prelu,
                                bias=bi_col, scale=sc_col, alpha=SLOPE,
                            )
                        else:
                            # (b) ScalarE drains PSUM: full-width
                            # Prelu(affine) -> bf16, then VectorE pools
                            yw = pmp.tile([C1, W2T], bf16, tag="yw")
                            nc.scalar.activation(
                                yw[:], zp2[:], Act.Prelu,
                                bias=bi_col, scale=sc_col, alpha=SLOPE,
                            )
                            nc.vector.tensor_tensor(
                                ot[:, WT : 2 * WT],
                                yw[:, 0:W2T:2], yw[:, 1:W2T:2], Alu.max,
                            )
                        # ship each 512-col half as soon as it's final; the
                        # final tiles ride the fast SWDGE queue so the tail
                        # is not ring-bandwidth-bound
                        i = ndma[0]
                        ndma[0] += 1
                        eng = (nc.sync, nc.scalar)[i % 2] if i < 8 else nc.gpsimd
                        eng.dma_start(
                            out=o_ap[s, C1 * c : C1 * (c + 1), WT * d : WT * (d + 1)],
                            in_=ot[:, WT * d : WT * (d + 1)],
                        )

            # interleaved emission: PE queue never waits on routing chains,
            # every drain follows its scale/bias producer in program order,
            # and block-2 (with its out DMAs) runs 1 sample behind block-1
            zp0 = emit_b1_mms(0)
            warm_mms(3)
            emit_rt1(0)
            warm_mms(5)
            emit_b1_drains(0, zp0)
            zp1 = emit_b1_mms(1)
            emit_rt2(0)
            emit_rt1(1)
            emit_b1_drains(1, zp1)
            emit_block2(0)
            zp2 = emit_b1_mms(2)
            emit_rt2(1)
            emit_rt1(2)
            emit_b1_drains(2, zp2)
            emit_block2(1)
            zp3 = emit_b1_mms(3)
            emit_rt2(2)
            emit_rt1(3)
            emit_b1_drains(3, zp3)
            emit_block2(2)
            emit_rt2(3)
            emit_block2(3)

    nc.compile()
    _built = nc
    return nc


def _pack_inputs(x, w1, b1, fcw1, fcb1, g1, be1, rm1, rv1,
                 w2, b2, fcw2, fcb2, g2, be2, rm2, rv2):
    import ml_dtypes

    f = np.float32
    bf = np.dtype(ml_dtypes.bfloat16)
    s1 = (g1 / np.sqrt(rv1 + EPS)).astype(f)
    s2 = (g2 / np.sqrt(rv2 + EPS)).astype(f)
    t11, t21 = (b1[0] * s1).astype(f), (be1 - rm1 * s1).astype(f)
    t12, t22 = (b2[0] * s2).astype(f), (be2 - rm2 * s2).astype(f)

    # bf16 weights: w1 taps01 / tap2 lhsT, routing fc (replicated), w2 lhsT
    w1t = w1[0].transpose(1, 2, 0).astype(f)           # (CIN, 3, C1)
    w2t = w2[0].transpose(1, 2, 0).reshape(C1, 3 * C2).astype(f)
    wpk = np.zeros((C1, NWB), bf)
    wpk[0:CIN, 0:C1] = w1t[:, 0, :].astype(bf)
    wpk[CIN:C1, 0:C1] = w1t[:, 1, :].astype(bf)
    wpk[0:CIN, T2 : T2 + C1] = w1t[:, 2, :].astype(bf)
    wpk[:, FC1OFF : FC1OFF + C1] = (fcw1[0] / W)[:, None].astype(bf)[
        np.arange(C1) % CIN
    ]
    wpk[:, FC2OFF : FC2OFF + C1] = (fcw2[0] / W)[:, None].astype(bf)
    wpk[:, W2OFF:NWB] = w2t.astype(bf)

    # fp32 fused-affine consts
    cpk = np.zeros((C1, NCC), f)
    cols = [s1, t11, t21, s2[:C1], s2[C1:], t12[:C1], t12[C1:],
            t22[:C1], t22[C1:], np.full(C1, fcb1[0], f), np.full(C1, fcb2[0], f)]
    for j, col in enumerate(cols):
        cpk[:, j] = col
    cpk[:, NCV : NCV + C1] = (fcw2[0] / W)[:, None]

    # x duplicated into partitions 64..127 shifted left by one column, so
    # block-1 taps 0+1 contract in a single K=128 matmul
    xb = x.astype(bf)
    xpk = np.zeros((B, C1, W + 2), bf)
    xpk[:, 0:CIN, 1 : W + 1] = xb
    xpk[:, CIN:C1, 0:W] = xb

    com = {"wpk": wpk, "cpk": cpk}
    return [
        {**com, "xpk": np.ascontiguousarray(xpk[i * BL : (i + 1) * BL])}
        for i in range(N_CORES)
    ]


def _enable_trace():
    """Register the NTFF profile hook (absent antenv.axon_hooks on this image)
    and stub out the S3 artifact upload so trace=True works locally."""
    import sys
    import types

    import concourse.bass_utils as bu

    bu.upload_artifacts = lambda tmpdir: tmpdir
    if "antenv.axon_hooks" not in sys.modules:
        import antenv
        from trn_agent_boot.trn_boot import _ntff_profile_via_ctypes

        hooks = types.ModuleType("antenv.axon_hooks")
        _store = {"hook": _ntff_profile_via_ctypes("/opt/axon/libaxon_pjrt.so")}
        hooks.set_axon_ntff_profile_hook = lambda h: _store.__setitem__("hook", h)
        hooks.get_axon_ntff_profile_hook = lambda: _store["hook"]
        sys.modules["antenv.axon_hooks"] = hooks
        antenv.axon_hooks = hooks


def kernel(**inputs):
    global LAST_RESULT
    from concourse.bass_utils import run_bass_kernel_spmd

    if TRACE:
        _enable_trace()
    nc = _build()
    in_maps = _pack_inputs(**inputs)
    res = run_bass_kernel_spmd(nc, in_maps, list(range(N_CORES)), trace=TRACE)
    LAST_RESULT = res
    return np.concatenate(
        [r["out"].astype(np.float32) for r in res.results], axis=0
    )


# revision 21
# speedup vs baseline: 1.0029x; 1.0029x over previous
"""Trainium2 Bass/Tile kernel for BasicCondConvBlock (E=1):
two CondConv1d(k=3,pad=1)+BN(eval)+LeakyReLU(0.1) blocks + MaxPool1d(2).

With a single expert, CondConv reduces to y_i = r_i * (conv(x_i, W) + b)
with a shared weight; routing r_i + conv bias + BatchNorm fold into one
per-(sample,channel) affine applied at PSUM-drain time:
    out = LeakyReLU( (r_i*s_c) * z + (r_i*b_c*s_c + be_c - rm_c*s_c) )

Everything off the fp32 PSUM accumulate runs in bf16 (tolerance 2e-2;
measured ~5e-3): conv matmuls (1 col/cycle vs ~2 for fp32r, weights get
FastWeightLoad), routing sums/matmuls (2-byte DVE ops hit 2x mode), and the
final output (halves the out-DMA bytes; the host converts back to f32).
Block-1 packs taps 0+1 into one K=128 matmul: x is host-duplicated into
partitions 64..127 shifted by one column, so each 512-col chunk is 2
matmuls (taps01 K=128 + tap2 K=64) instead of 3.

PE-queue discipline (the engine queue is in-order, so anything emitted
before conv matmuls stalls them): a zero-tile warm-up burst at t=0 covers
the x0-DMA latency and warms the HAM clock gate to full speed (a cold PE
runs at 1.2 GHz; one >3us idle gap mid-kernel re-throttles it, so the x
DMAs are spread over the SWDGE + sync + scalar queues to keep the conv
stream fed).  Each routing matmul is emitted AFTER the conv matmuls that
hide its DVE-reduce latency; block-2 of sample s is interleaved right
after block-1 of sample s+1 so out-DMAs spread across the whole kernel
instead of piling into a tail.

The per-sample x row-sum is two half-width DVE reduces (top partitions sum
the first half of x, the shifted bottom copy sums the second half); the
routing matmul's replicated fc weight spans all 128 partitions so the
contraction adds the two partials for free.

Drains: block-1 writes y1 as bf16 via ScalarE Prelu(affine) per [128,1024]
PSUM tile with fused row-sum (feeds block-2 routing); one tile drains on
VectorE.  Block-2 drains 10-of-16 tiles as (a) VectorE 3D-AP max-pool
PSUM->bf16 then half-width ScalarE Prelu(affine) (exact: scale>0 keeps
affine+Prelu monotone) and 6-of-16 as (b) full-width ScalarE
Prelu(affine)->bf16 then VectorE strided pool, splitting PSUM reads across
engines.  Sharding: pure data parallel over batch (32 -> 4x8).
"""

import numpy as np

N_CORES = 8
B, CIN, W = 32, 64, 2048
C1, C2 = 128, 256
BL = B // N_CORES  # samples per core
EPS = 1e-5
SLOPE = 0.1
WT = 512           # conv output tile width (one PSUM bank of fp32)
WO = W // 2        # pooled output width
W2T = 2 * WT       # PSUM tiles span two banks; drains amortize per-op overhead
NWARM = 14         # HAM warm-up matmuls; bridges the x0 DMA (no PE idle)

# wpk (bf16) column layout: [0:128] w1 taps01 lhsT (rows 0-63 tap0, 64-127
# tap1), [128:256] w1 tap2 (rows 0-63), [256:384] fcw1/W replicated over all
# rows (the rt1 matmul contracts the two half-sum partials), [384:512]
# fcw2/W replicated, [512:1280] w2 lhsT [i, k*C2+o]
T2 = 128
FC1OFF = 256
FC2OFF = 384
WAEND = 512
W2OFF = 512
NWB = W2OFF + 3 * C2
# cpk (f32) consts: s1, t11, t21, s2a, s2b, t12a, t12b, t22a, t22b, fcb1,
# fcb2
NCC = 11

TRACE = False
LAST_RESULT = None

_built = None


def _build():
    global _built
    if _built is not None:
        return _built

    import concourse.bacc as bacc
    import concourse.mybir as mybir
    from concourse import tile
    from contextlib import ExitStack

    f32 = mybir.dt.float32
    bf16 = mybir.dt.bfloat16
    Alu = mybir.AluOpType
    Act = mybir.ActivationFunctionType
    Ax = mybir.AxisListType

    nc = bacc.Bacc("TRN2", target_bir_lowering=False, debug=False)

    xd = nc.declare_dram_parameter("xpk", [BL, C1, W + 2], bf16, isOutput=False)
    wd = nc.declare_dram_parameter("wpk", [C1, NWB], bf16, isOutput=False)
    cd = nc.declare_dram_parameter("cpk", [C1, NCC], f32, isOutput=False)
    od = nc.declare_dram_parameter("out", [BL, C2, WO], bf16, isOutput=True)
    x_ap, w_ap, c_ap, o_ap = xd.ap(), wd.ap(), cd.ap(), od.ap()

    with tile.TileContext(nc) as tc:
        with ExitStack() as ctx:
            consts = ctx.enter_context(tc.tile_pool(name="consts", bufs=1))
            xpool = ctx.enter_context(tc.tile_pool(name="xp", bufs=BL))
            y1pool = ctx.enter_context(tc.tile_pool(name="y1p", bufs=BL))
            pmp = ctx.enter_context(tc.tile_pool(name="pmp", bufs=6))
            outp = ctx.enter_context(tc.tile_pool(name="outp", bufs=8))
            small = ctx.enter_context(tc.tile_pool(name="small", bufs=1))
            psum = ctx.enter_context(tc.tile_pool(name="psum", bufs=3, space="PSUM"))
            psmall = ctx.enter_context(tc.tile_pool(name="psm", bufs=1, space="PSUM"))
            pwarm = ctx.enter_context(tc.tile_pool(name="pwm", bufs=1, space="PSUM"))

            # --- input DMAs.  SWDGE (gpsimd) is ~2x the ring rate: it
            # carries x0 first (gates the first conv), then x1's second half
            # and x3; the sync ring brings x1's first half and x2; the
            # scalar ring brings w1+routing weights, consts, then w2.
            xts = [
                xpool.tile([C1, W + 2], bf16, tag="xt", name=f"xt{s}")
                for s in range(BL)
            ]
            ws = consts.tile([C1, NWB], bf16)
            cs = consts.tile([C1, NCC], f32)
            nc.gpsimd.dma_start(out=xts[0][0:CIN], in_=x_ap[0][0:CIN])
            nc.gpsimd.dma_start(out=xts[1][0:CIN], in_=x_ap[1][0:CIN])
            nc.gpsimd.dma_start(out=xts[3][:], in_=x_ap[3])
            nc.sync.dma_start(out=xts[0][CIN:C1], in_=x_ap[0][CIN:C1])
            nc.sync.dma_start(out=xts[1][CIN:C1], in_=x_ap[1][CIN:C1])
            nc.sync.dma_start(out=xts[2][CIN:C1], in_=x_ap[2][CIN:C1])
            nc.scalar.dma_start(out=ws[:, 0:WAEND], in_=w_ap[:, 0:WAEND])
            nc.scalar.dma_start(out=cs[:], in_=c_ap[:])
            nc.scalar.dma_start(out=ws[:, W2OFF:], in_=w_ap[:, W2OFF:])
            nc.scalar.dma_start(out=xts[2][0:CIN], in_=x_ap[2][0:CIN])

            def ck(j):
                return cs[:, j : j + 1]

            # --- HAM warm-up: zero-tile matmuls keep the PE busy while the
            # x DMAs land, so the array is at full clock for the real work.
            wz = consts.tile([C1, 640], bf16)
            nc.vector.memset(wz[:], 0.0)
            zpw = pwarm.tile([C1, WT], f32, name="warm")

            def warm_mms(n):
                for _ in range(n):
                    nc.tensor.matmul(
                        zpw[:], wz[:, 0:C1], wz[:, C1 : C1 + WT],
                        start=True, stop=True,
                    )

            warm_mms(NWARM)
            # dummy activations preload both ScalarE function tables during
            # the DMA dead time (a mid-kernel ACT_TABLE_LOAD costs 1.3us on
            # the routing critical path)
            tscr = small.tile([C1, 2], f32)
            nc.scalar.activation(tscr[:, 0:1], wz[:, 0:1], Act.Sigmoid,
                                 bias=0.0, scale=1.0)
            nc.scalar.activation(tscr[:, 1:2], wz[:, 0:1], Act.Prelu,
                                 bias=0.0, scale=1.0, alpha=SLOPE)

            m1 = small.tile([C1, BL], bf16)
            rbc1 = small.tile([C1, BL], f32)
            sc1 = small.tile([C1, BL], f32)
            bi1 = small.tile([C1, BL], f32)
            s1acc = small.tile([C1, 2 * BL], f32)
            ssum = small.tile([C1, BL], bf16)
            gscr = [small.tile([C1, W // 2], f32, name=f"gscr{s}") for s in range(2)]
            rbc2 = small.tile([C1, BL], f32)
            sc2 = small.tile([C1, 2 * BL], f32)
            bi2 = small.tile([C1, 2 * BL], f32)

            y1s = [
                y1pool.tile([C1, W + 2], bf16, tag="y1", name=f"y1_{s}")
                for s in range(BL)
            ]

            def emit_b1_mms(s):
                """Conv1 matmuls (PSUM fills) + routing row-sums, sample s."""
                xt = xts[s]
                y1 = y1s[s]
                nc.vector.memset(y1[:, 0 : W + 2 : W + 1], 0.0)
                with nc.allow_low_precision("routing logit tolerates bf16"):
                    if s < 2:
                        # two DVE half-reduces (latency-critical early
                        # samples); the top read starts at the zero pad so
                        # it misses x[1023], added back by a 1-element op
                        nc.vector.reduce_sum(
                            m1[0:CIN, s : s + 1], xt[0:CIN, 0 : W // 2], axis=Ax.X
                        )
                        nc.vector.reduce_sum(
                            m1[CIN:C1, s : s + 1], xt[CIN:C1, W // 2 : W], axis=Ax.X
                        )
                        nc.vector.tensor_tensor(
                            m1[0:CIN, s : s + 1], m1[0:CIN, s : s + 1],
                            xt[0:CIN, W // 2 : W // 2 + 1], Alu.add,
                        )
                    else:
                        # GpSimd folds the bottom (shifted) copy, which
                        # covers x exactly; one short DVE reduce finishes.
                        # Keeps the big reduces off the DVE during its
                        # busiest stretch.
                        g = gscr[s - 2]
                        nc.gpsimd.tensor_tensor(
                            g[CIN:C1, :], xt[CIN:C1, 0 : W // 2],
                            xt[CIN:C1, W // 2 : W], Alu.add,
                        )
                        nc.vector.reduce_sum(
                            m1[CIN:C1, s : s + 1], g[CIN:C1, :], axis=Ax.X
                        )
                zps = []
                for d in range(2):
                    zp = psum.tile([C1, W2T], f32, tag="zp")
                    # weight-major order: consecutive matmuls share the
                    # stationary operand, halving effective LDWEIGHTS
                    for h in range(2):
                        t0 = W2T * d + WT * h
                        nc.tensor.matmul(
                            zp[:, WT * h : WT * h + WT], ws[:, 0:C1],
                            xt[:, t0 : t0 + WT], start=True, stop=False,
                        )
                    for h in range(2):
                        t0 = W2T * d + WT * h
                        nc.tensor.matmul(
                            zp[:, WT * h : WT * h + WT], ws[0:CIN, T2 : T2 + C1],
                            xt[0:CIN, t0 + 2 : t0 + 2 + WT],
                            start=False, stop=True,
                        )
                    zps.append(zp)
                return zps

            def emit_b1_drains(s, zps):
                """Block-1 PSUM drains for sample s.  Emitted after
                emit_rt1(s) so the scale/bias reads follow their writers in
                program order (the tile scheduler takes deps from it)."""
                y1 = y1s[s]
                for d in range(2):
                    acc = s1acc[:, 2 * s + d : 2 * s + d + 1]
                    dst = y1[:, 1 + W2T * d : 1 + W2T * (d + 1)]
                    if not (s == 3 and d == 1):
                        # ScalarE drain: fused Prelu(affine) + row-sum
                        nc.scalar.activation(
                            dst, zps[d][:], Act.Prelu,
                            bias=bi1[:, s : s + 1], scale=sc1[:, s : s + 1],
                            alpha=SLOPE, accum_out=acc,
                        )
                    else:
                        # one VectorE drain balances late-kernel ScalarE load
                        ytmp = pmp.tile([C1, W2T], bf16, tag="ytmp")
                        nc.vector.tensor_scalar(
                            ytmp[:], zps[d][:],
                            sc1[:, s : s + 1], bi1[:, s : s + 1],
                            Alu.mult, Alu.add,
                        )
                        nc.vector.scalar_tensor_tensor(
                            dst, ytmp[:], SLOPE, ytmp[:], Alu.mult, Alu.max,
                            accum_out=acc,
                        )

            def emit_rt1(s):
                """Routing-1 matmul + sigmoid + scale/bias cols for sample s.
                Emitted after conv matmuls that hide the m1 reduce latency."""
                lgb = psmall.tile([C1, 1], f32, tag="sm", name=f"lg1{s}")
                if s < 2:
                    nc.tensor.matmul(
                        lgb[:], ws[:, FC1OFF : FC1OFF + C1], m1[:, s : s + 1],
                        start=True, stop=True,
                    )
                else:
                    nc.tensor.matmul(
                        lgb[:], ws[CIN:C1, FC1OFF : FC1OFF + C1],
                        m1[CIN:C1, s : s + 1], start=True, stop=True,
                    )
                nc.scalar.activation(
                    rbc1[:, s : s + 1], lgb[:], Act.Sigmoid, bias=ck(9), scale=1.0
                )
                nc.vector.tensor_scalar(
                    sc1[:, s : s + 1], ck(0), rbc1[:, s : s + 1], None, Alu.mult
                )
                nc.vector.scalar_tensor_tensor(
                    bi1[:, s : s + 1], ck(1), rbc1[:, s : s + 1], ck(2),
                    Alu.mult, Alu.add,
                )

            def emit_rt2(s):
                """Routing-2 chain for sample s, gated on its block-1 drains."""
                with nc.allow_low_precision("routing logit tolerates bf16"):
                    nc.vector.reduce_sum(
                        ssum[:, s : s + 1], s1acc[:, 2 * s : 2 * (s + 1)], axis=Ax.X
                    )
                lgb2 = psmall.tile([C1, 1], f32, tag="sm", name=f"lg2{s}")
                nc.tensor.matmul(
                    lgb2[:], ws[:, FC2OFF : FC2OFF + C1], ssum[:, s : s + 1],
                    start=True, stop=True,
                )
                nc.scalar.activation(
                    rbc2[:, s : s + 1], lgb2[:], Act.Sigmoid, bias=ck(10), scale=1.0
                )
                # both c-chunks in one paired op (consts are adjacent cols)
                nc.vector.tensor_scalar(
                    sc2[:, 2 * s : 2 * s + 2], cs[:, 3:5],
                    rbc2[:, s : s + 1], None, Alu.mult,
                )
                nc.vector.scalar_tensor_tensor(
                    bi2[:, 2 * s : 2 * s + 2], cs[:, 5:7],
                    rbc2[:, s : s + 1], cs[:, 7:9], Alu.mult, Alu.add,
                )

            ndma = [0]

            def emit_block2(s):
                """Conv2 (24 MMs) + alternating drains + out DMAs, sample s."""
                for c in range(2):
                    ot = outp.tile([C1, WO], bf16, tag="ot")
                    sc_col = sc2[:, 2 * s + c : 2 * s + c + 1]
                    bi_col = bi2[:, 2 * s + c : 2 * s + c + 1]
                    for d in range(2):
                        zp2 = psum.tile([C1, W2T], f32, tag="zp")
                        for k in range(3):
                            for h in range(2):
                                t0 = W2T * d + WT * h
                                nc.tensor.matmul(
                                    zp2[:, WT * h : WT * h + WT],
                                    ws[:, W2OFF + k * C2 + C1 * c : W2OFF + k * C2 + C1 * c + C1],
                                    y1s[s][:, t0 + k : t0 + k + WT],
                                    start=(k == 0), stop=(k == 2),
                                )
                        if d == 0 or (s, c) in ((1, 0), (3, 0)):
                            # (a) VectorE drains PSUM: one-input 3D-AP
                            # max-pool -> bf16, then ScalarE Prelu(affine) at
                            # half width (exact: scale>0 keeps it monotone)
                            pm = pmp.tile([C1, WT], bf16, tag="pm")
                            nc.vector.tensor_reduce(
                                pm[:], zp2[:].rearrange("p (a b) -> p a b", b=2),
                                axis=Ax.X, op=Alu.max,
                            )
                            nc.scalar.activation(
                                ot[:, WT * d : WT * (d + 1)], pm[:], Act.Prelu,
                                bias=bi_col, scale=sc_col, alpha=SLOPE,
                            )
                        else:
                            # (b) ScalarE drains PSUM: full-width
                            # Prelu(affine) -> bf16, then VectorE pools
                            yw = pmp.tile([C1, W2T], bf16, tag="yw")
                            nc.scalar.activation(
                                yw[:], zp2[:], Act.Prelu,
                                bias=bi_col, scale=sc_col, alpha=SLOPE,
                            )
                            nc.vector.tensor_tensor(
                                ot[:, WT : 2 * WT],
                                yw[:, 0:W2T:2], yw[:, 1:W2T:2], Alu.max,
                            )
                        # ship each 512-col half as soon as it's final; the
                        # final tiles ride the fast SWDGE queue so the tail
                        # is not ring-bandwidth-bound
                        i = ndma[0]
                        ndma[0] += 1
                        eng = (nc.sync, nc.scalar)[i % 2] if i < 8 else nc.gpsimd
                        eng.dma_start(
                            out=o_ap[s, C1 * c : C1 * (c + 1), WT * d : WT * (d + 1)],
                            in_=ot[:, WT * d : WT * (d + 1)],
                        )

            # interleaved emission: PE queue never waits on routing chains,
            # every drain follows its scale/bias producer in program order,
            # and block-2 (with its out DMAs) runs 1 sample behind block-1
            zp0 = emit_b1_mms(0)
            warm_mms(3)
            emit_rt1(0)
            warm_mms(5)
            emit_b1_drains(0, zp0)
            zp1 = emit_b1_mms(1)
            emit_rt2(0)
            emit_rt1(1)
            emit_b1_drains(1, zp1)
            emit_block2(0)
            zp2 = emit_b1_mms(2)
            emit_rt2(1)
            emit_rt1(2)
            emit_b1_drains(2, zp2)
            emit_block2(1)
            zp3 = emit_b1_mms(3)
            emit_rt2(2)
            emit_rt1(3)
            emit_b1_drains(3, zp3)
            emit_block2(2)
            emit_rt2(3)
            emit_block2(3)

    nc.compile()
    _built = nc
    return nc


def _pack_inputs(x, w1, b1, fcw1, fcb1, g1, be1, rm1, rv1,
                 w2, b2, fcw2, fcb2, g2, be2, rm2, rv2):
    import ml_dtypes

    f = np.float32
    bf = np.dtype(ml_dtypes.bfloat16)
    s1 = (g1 / np.sqrt(rv1 + EPS)).astype(f)
    s2 = (g2 / np.sqrt(rv2 + EPS)).astype(f)
    t11, t21 = (b1[0] * s1).astype(f), (be1 - rm1 * s1).astype(f)
    t12, t22 = (b2[0] * s2).astype(f), (be2 - rm2 * s2).astype(f)

    # bf16 weights: w1 taps01 / tap2 lhsT, routing fc (replicated), w2 lhsT
    w1t = w1[0].transpose(1, 2, 0).astype(f)           # (CIN, 3, C1)
    w2t = w2[0].transpose(1, 2, 0).reshape(C1, 3 * C2).astype(f)
    wpk = np.zeros((C1, NWB), bf)
    wpk[0:CIN, 0:C1] = w1t[:, 0, :].astype(bf)
    wpk[CIN:C1, 0:C1] = w1t[:, 1, :].astype(bf)
    wpk[0:CIN, T2 : T2 + C1] = w1t[:, 2, :].astype(bf)
    wpk[:, FC1OFF : FC1OFF + C1] = (fcw1[0] / W)[:, None].astype(bf)[
        np.arange(C1) % CIN
    ]
    wpk[:, FC2OFF : FC2OFF + C1] = (fcw2[0] / W)[:, None].astype(bf)
    wpk[:, W2OFF:NWB] = w2t.astype(bf)

    # fp32 fused-affine consts
    cpk = np.zeros((C1, NCC), f)
    cols = [s1, t11, t21, s2[:C1], s2[C1:], t12[:C1], t12[C1:],
            t22[:C1], t22[C1:], np.full(C1, fcb1[0], f), np.full(C1, fcb2[0], f)]
    for j, col in enumerate(cols):
        cpk[:, j] = col

    # x duplicated into partitions 64..127 shifted left by one column, so
    # block-1 taps 0+1 contract in a single K=128 matmul
    xb = x.astype(bf)
    xpk = np.zeros((B, C1, W + 2), bf)
    xpk[:, 0:CIN, 1 : W + 1] = xb
    xpk[:, CIN:C1, 0:W] = xb

    com = {"wpk": wpk, "cpk": cpk}
    return [
        {**com, "xpk": np.ascontiguousarray(xpk[i * BL : (i + 1) * BL])}
        for i in range(N_CORES)
    ]


def _enable_trace():
    """Register the NTFF profile hook (absent antenv.axon_hooks on this image)
    and stub out the S3 artifact upload so trace=True works locally."""
    import sys
    import types

    import concourse.bass_utils as bu

    bu.upload_artifacts = lambda tmpdir: tmpdir
    if "antenv.axon_hooks" not in sys.modules:
        import antenv
        from trn_agent_boot.trn_boot import _ntff_profile_via_ctypes

        hooks = types.ModuleType("antenv.axon_hooks")
        _store = {"hook": _ntff_profile_via_ctypes("/opt/axon/libaxon_pjrt.so")}
        hooks.set_axon_ntff_profile_hook = lambda h: _store.__setitem__("hook", h)
        hooks.get_axon_ntff_profile_hook = lambda: _store["hook"]
        sys.modules["antenv.axon_hooks"] = hooks
        antenv.axon_hooks = hooks


def kernel(**inputs):
    global LAST_RESULT
    from concourse.bass_utils import run_bass_kernel_spmd

    if TRACE:
        _enable_trace()
    nc = _build()
    in_maps = _pack_inputs(**inputs)
    res = run_bass_kernel_spmd(nc, in_maps, list(range(N_CORES)), trace=TRACE)
    LAST_RESULT = res
    return np.concatenate(
        [r["out"].astype(np.float32) for r in res.results], axis=0
    )


# revision 22
# speedup vs baseline: 1.0122x; 1.0093x over previous
"""Trainium2 Bass/Tile kernel for BasicCondConvBlock (E=1):
two CondConv1d(k=3,pad=1)+BN(eval)+LeakyReLU(0.1) blocks + MaxPool1d(2).

With a single expert, CondConv reduces to y_i = r_i * (conv(x_i, W) + b)
with a shared weight; routing r_i + conv bias + BatchNorm fold into one
per-(sample,channel) affine applied at PSUM-drain time:
    out = LeakyReLU( (r_i*s_c) * z + (r_i*b_c*s_c + be_c - rm_c*s_c) )

Everything off the fp32 PSUM accumulate runs in bf16 (tolerance 2e-2;
measured ~5e-3): conv matmuls (1 col/cycle vs ~2 for fp32r, weights get
FastWeightLoad), routing sums/matmuls, and the final output (halves the
out-DMA bytes; the host converts back to f32).  Block-1 packs taps 0+1
into one K=128 matmul: x is host-duplicated into partitions 64..127
shifted by one column, so each 512-col chunk is 2 matmuls (taps01 K=128 +
tap2 K=64) instead of 3.  Matmuls issue weight-major so consecutive ones
share the stationary operand.

PE-queue discipline (the engine queue is in-order, so anything emitted
before conv matmuls stalls them): a zero-tile warm-up burst at t=0 covers
the x0-DMA latency and warms the HAM clock gate to full speed (a cold PE
runs at 1.2 GHz; a >~2us idle gap mid-kernel re-throttles it, costing
~3.4us of half-clock to re-warm); dummy Sigmoid/Prelu activations preload
both ScalarE function tables during the same dead time (a mid-kernel
ACT_TABLE_LOAD is 1.3us on the routing critical path).  Each routing
matmul is emitted AFTER the conv matmuls that hide its reduce latency,
with a couple of filler warm matmuls bridging the s0 routing lull;
block-2 of sample s runs 1 sample behind block-1 so out-DMAs spread
across the whole kernel instead of piling into a tail.

IMPORTANT emission-order invariant: the Tile scheduler takes dependencies
from program order, so every PSUM drain must be emitted AFTER the routing
ops that produce its scale/bias columns (emit_b1_mms / emit_rt1 /
emit_b1_drains are interleaved accordingly).  Violating this races the
drain against the routing chain; the result looks correct on repeat runs
(SBUF retains the previous run's identical columns) but is garbage on a
fresh first run.

Routing-1's per-sample x row-sum: samples 0/1 use two 4B-aligned DVE
half-reduces (top partitions sum the first half of x starting at the zero
pad, missing x[1023], added back by a 1-element op; the shifted bottom
copy sums the second half); the routing matmul's replicated fc weight
spans all 128 partitions so the contraction adds the partials for free.
Samples 2/3 instead fold on GpSimd (the shifted bottom copy covers x
exactly) + one short DVE reduce, keeping the big reduces off the DVE at
its busiest.  Routing-2 sums the two drain row-sum accumulators with a
GpSimd add (plain tensor_tensor is the only gpsimd op that survives
codegen with these operands).

Drains: block-1 writes y1 as bf16 via ScalarE Prelu(affine) per
[128,1024] PSUM tile with fused row-sum (feeds block-2 routing); the last
tile drains on VectorE.  Block-2 drains 10-of-16 tiles as (a) VectorE
3D-AP max-pool PSUM->bf16 then half-width ScalarE Prelu(affine) (exact:
scale>0 keeps affine+Prelu monotone) and 6-of-16 as (b) full-width
ScalarE Prelu(affine)->bf16 then VectorE strided pool, splitting PSUM
reads across engines.

DMA: x moves as partition-split halves (each DRAM-contiguous; strided
column-halves are ~3x slower) spread over the SWDGE + sync + scalar
queues so sample s arrives just before its conv; out tiles ship per
512-col half, early ones on the slow rings, late ones on SWDGE.
Sharding: pure data parallel over batch (32 -> 4x8).
"""

import numpy as np

N_CORES = 8
B, CIN, W = 32, 64, 2048
C1, C2 = 128, 256
BL = B // N_CORES  # samples per core
EPS = 1e-5
SLOPE = 0.1
WT = 512           # conv output tile width (one PSUM bank of fp32)
WO = W // 2        # pooled output width
W2T = 2 * WT       # PSUM tiles span two banks; drains amortize per-op overhead
NWARM = 14         # HAM warm-up matmuls; bridges the x0 DMA (no PE idle)

# wpk (bf16) column layout: [0:128] w1 taps01 lhsT (rows 0-63 tap0, 64-127
# tap1), [128:256] w1 tap2 (rows 0-63), [256:384] fcw1/W replicated over all
# rows (the rt1 matmul contracts the two half-sum partials), [384:512]
# fcw2/W replicated, [512:1280] w2 lhsT [i, k*C2+o]
T2 = 128
FC1OFF = 256
FC2OFF = 384
WAEND = 512
W2OFF = 512
NWB = W2OFF + 3 * C2
# cpk (f32) consts: s1, t11, t21, s2a, s2b, t12a, t12b, t22a, t22b, fcb1,
# fcb2
NCC = 11

TRACE = False
LAST_RESULT = None

_built = None


def _build():
    global _built
    if _built is not None:
        return _built

    import concourse.bacc as bacc
    import concourse.mybir as mybir
    from concourse import tile
    from contextlib import ExitStack

    f32 = mybir.dt.float32
    bf16 = mybir.dt.bfloat16
    Alu = mybir.AluOpType
    Act = mybir.ActivationFunctionType
    Ax = mybir.AxisListType

    nc = bacc.Bacc("TRN2", target_bir_lowering=False, debug=False)

    xd = nc.declare_dram_parameter("xpk", [BL, C1, W + 2], bf16, isOutput=False)
    wd = nc.declare_dram_parameter("wpk", [C1, NWB], bf16, isOutput=False)
    cd = nc.declare_dram_parameter("cpk", [C1, NCC], f32, isOutput=False)
    od = nc.declare_dram_parameter("out", [BL, C2, WO], bf16, isOutput=True)
    x_ap, w_ap, c_ap, o_ap = xd.ap(), wd.ap(), cd.ap(), od.ap()

    with tile.TileContext(nc) as tc:
        with ExitStack() as ctx:
            consts = ctx.enter_context(tc.tile_pool(name="consts", bufs=1))
            xpool = ctx.enter_context(tc.tile_pool(name="xp", bufs=BL))
            y1pool = ctx.enter_context(tc.tile_pool(name="y1p", bufs=BL))
            pmp = ctx.enter_context(tc.tile_pool(name="pmp", bufs=6))
            outp = ctx.enter_context(tc.tile_pool(name="outp", bufs=8))
            small = ctx.enter_context(tc.tile_pool(name="small", bufs=1))
            psum = ctx.enter_context(tc.tile_pool(name="psum", bufs=3, space="PSUM"))
            psmall = ctx.enter_context(tc.tile_pool(name="psm", bufs=1, space="PSUM"))
            pwarm = ctx.enter_context(tc.tile_pool(name="pwm", bufs=1, space="PSUM"))

            # --- input DMAs.  SWDGE (gpsimd) is ~2x the ring rate: it
            # carries x0 first (gates the first conv), then x1's second half
            # and x3; the sync ring brings x1's first half and x2; the
            # scalar ring brings w1+routing weights, consts, then w2.
            xts = [
                xpool.tile([C1, W + 2], bf16, tag="xt", name=f"xt{s}")
                for s in range(BL)
            ]
            ws = consts.tile([C1, NWB], bf16)
            cs = consts.tile([C1, NCC], f32)
            nc.gpsimd.dma_start(out=xts[0][0:CIN], in_=x_ap[0][0:CIN])
            nc.gpsimd.dma_start(out=xts[1][0:CIN], in_=x_ap[1][0:CIN])
            nc.gpsimd.dma_start(out=xts[3][:], in_=x_ap[3])
            nc.sync.dma_start(out=xts[0][CIN:C1], in_=x_ap[0][CIN:C1])
            nc.sync.dma_start(out=xts[1][CIN:C1], in_=x_ap[1][CIN:C1])
            nc.sync.dma_start(out=xts[2][CIN:C1], in_=x_ap[2][CIN:C1])
            nc.scalar.dma_start(out=ws[:, 0:WAEND], in_=w_ap[:, 0:WAEND])
            nc.scalar.dma_start(out=cs[:], in_=c_ap[:])
            nc.scalar.dma_start(out=ws[:, W2OFF:], in_=w_ap[:, W2OFF:])
            nc.scalar.dma_start(out=xts[2][0:CIN], in_=x_ap[2][0:CIN])

            def ck(j):
                return cs[:, j : j + 1]

            # --- HAM warm-up: zero-tile matmuls keep the PE busy while the
            # x DMAs land, so the array is at full clock for the real work.
            wz = consts.tile([C1, 640], bf16)
            nc.vector.memset(wz[:], 0.0)
            zpw = pwarm.tile([C1, WT], f32, name="warm")

            def warm_mms(n):
                for _ in range(n):
                    nc.tensor.matmul(
                        zpw[:], wz[:, 0:C1], wz[:, C1 : C1 + WT],
                        start=True, stop=True,
                    )

            warm_mms(NWARM)
            # dummy activations preload both ScalarE function tables during
            # the DMA dead time (a mid-kernel ACT_TABLE_LOAD costs 1.3us on
            # the routing critical path)
            tscr = small.tile([C1, 2], f32)
            nc.scalar.activation(tscr[:, 0:1], wz[:, 0:1], Act.Sigmoid,
                                 bias=0.0, scale=1.0)
            nc.scalar.activation(tscr[:, 1:2], wz[:, 0:1], Act.Prelu,
                                 bias=0.0, scale=1.0, alpha=SLOPE)

            m1 = small.tile([C1, BL], bf16)
            rbc1 = small.tile([C1, BL], f32)
            sc1 = small.tile([C1, BL], f32)
            bi1 = small.tile([C1, BL], f32)
            s1acc = small.tile([C1, 2 * BL], f32)
            ssum = small.tile([C1, BL], bf16)
            gscr = [small.tile([C1, W // 2], f32, name=f"gscr{s}") for s in range(2)]
            rbc2 = small.tile([C1, BL], f32)
            sc2 = small.tile([C1, 2 * BL], f32)
            bi2 = small.tile([C1, 2 * BL], f32)

            y1s = [
                y1pool.tile([C1, W + 2], bf16, tag="y1", name=f"y1_{s}")
                for s in range(BL)
            ]

            def emit_b1_mms(s):
                """Conv1 matmuls (PSUM fills) + routing row-sums, sample s."""
                xt = xts[s]
                y1 = y1s[s]
                nc.vector.memset(y1[:, 0 : W + 2 : W + 1], 0.0)
                with nc.allow_low_precision("routing logit tolerates bf16"):
                    if s < 2:
                        # two DVE half-reduces (latency-critical early
                        # samples); the top read starts at the zero pad so
                        # it misses x[1023], added back by a 1-element op
                        nc.vector.reduce_sum(
                            m1[0:CIN, s : s + 1], xt[0:CIN, 0 : W // 2], axis=Ax.X
                        )
                        nc.vector.reduce_sum(
                            m1[CIN:C1, s : s + 1], xt[CIN:C1, W // 2 : W], axis=Ax.X
                        )
                        nc.vector.tensor_tensor(
                            m1[0:CIN, s : s + 1], m1[0:CIN, s : s + 1],
                            xt[0:CIN, W // 2 : W // 2 + 1], Alu.add,
                        )
                    else:
                        # GpSimd folds the bottom (shifted) copy, which
                        # covers x exactly; one short DVE reduce finishes.
                        # Keeps the big reduces off the DVE during its
                        # busiest stretch.
                        g = gscr[s - 2]
                        nc.gpsimd.tensor_tensor(
                            g[CIN:C1, :], xt[CIN:C1, 0 : W // 2],
                            xt[CIN:C1, W // 2 : W], Alu.add,
                        )
                        nc.vector.reduce_sum(
                            m1[CIN:C1, s : s + 1], g[CIN:C1, :], axis=Ax.X
                        )
                zps = []
                for d in range(2):
                    zp = psum.tile([C1, W2T], f32, tag="zp")
                    # weight-major order: consecutive matmuls share the
                    # stationary operand, halving effective LDWEIGHTS
                    for h in range(2):
                        t0 = W2T * d + WT * h
                        nc.tensor.matmul(
                            zp[:, WT * h : WT * h + WT], ws[:, 0:C1],
                            xt[:, t0 : t0 + WT], start=True, stop=False,
                        )
                    for h in range(2):
                        t0 = W2T * d + WT * h
                        nc.tensor.matmul(
                            zp[:, WT * h : WT * h + WT], ws[0:CIN, T2 : T2 + C1],
                            xt[0:CIN, t0 + 2 : t0 + 2 + WT],
                            start=False, stop=True,
                        )
                    zps.append(zp)
                return zps

            def emit_b1_drains(s, zps):
                """Block-1 PSUM drains for sample s.  Emitted after
                emit_rt1(s) so the scale/bias reads follow their writers in
                program order (the tile scheduler takes deps from it)."""
                y1 = y1s[s]
                for d in range(2):
                    acc = s1acc[:, 2 * s + d : 2 * s + d + 1]
                    dst = y1[:, 1 + W2T * d : 1 + W2T * (d + 1)]
                    if not (s == 3 and d == 1):
                        # ScalarE drain: fused Prelu(affine) + row-sum
                        nc.scalar.activation(
                            dst, zps[d][:], Act.Prelu,
                            bias=bi1[:, s : s + 1], scale=sc1[:, s : s + 1],
                            alpha=SLOPE, accum_out=acc,
                        )
                    else:
                        # one VectorE drain balances late-kernel ScalarE load
                        ytmp = pmp.tile([C1, W2T], bf16, tag="ytmp")
                        nc.vector.tensor_scalar(
                            ytmp[:], zps[d][:],
                            sc1[:, s : s + 1], bi1[:, s : s + 1],
                            Alu.mult, Alu.add,
                        )
                        nc.vector.scalar_tensor_tensor(
                            dst, ytmp[:], SLOPE, ytmp[:], Alu.mult, Alu.max,
                            accum_out=acc,
                        )

            def emit_rt1(s):
                """Routing-1 matmul + sigmoid + scale/bias cols for sample s.
                Emitted after conv matmuls that hide the m1 reduce latency."""
                lgb = psmall.tile([C1, 1], f32, tag="sm", name=f"lg1{s}")
                if s < 2:
                    nc.tensor.matmul(
                        lgb[:], ws[:, FC1OFF : FC1OFF + C1], m1[:, s : s + 1],
                        start=True, stop=True,
                    )
                else:
                    nc.tensor.matmul(
                        lgb[:], ws[CIN:C1, FC1OFF : FC1OFF + C1],
                        m1[CIN:C1, s : s + 1], start=True, stop=True,
                    )
                nc.scalar.activation(
                    rbc1[:, s : s + 1], lgb[:], Act.Sigmoid, bias=ck(9), scale=1.0
                )
                nc.vector.tensor_scalar(
                    sc1[:, s : s + 1], ck(0), rbc1[:, s : s + 1], None, Alu.mult
                )
                nc.vector.scalar_tensor_tensor(
                    bi1[:, s : s + 1], ck(1), rbc1[:, s : s + 1], ck(2),
                    Alu.mult, Alu.add,
                )

            def emit_rt2(s):
                """Routing-2 chain for sample s, gated on its block-1 drains."""
                with nc.allow_low_precision("routing logit tolerates bf16"):
                    nc.vector.reduce_sum(
                        ssum[:, s : s + 1], s1acc[:, 2 * s : 2 * (s + 1)], axis=Ax.X
                    )
                lgb2 = psmall.tile([C1, 1], f32, tag="sm", name=f"lg2{s}")
                nc.tensor.matmul(
                    lgb2[:], ws[:, FC2OFF : FC2OFF + C1], ssum[:, s : s + 1],
                    start=True, stop=True,
                )
                nc.scalar.activation(
                    rbc2[:, s : s + 1], lgb2[:], Act.Sigmoid, bias=ck(10), scale=1.0
                )
                # both c-chunks in one paired op (consts are adjacent cols)
                nc.vector.tensor_scalar(
                    sc2[:, 2 * s : 2 * s + 2], cs[:, 3:5],
                    rbc2[:, s : s + 1], None, Alu.mult,
                )
                nc.vector.scalar_tensor_tensor(
                    bi2[:, 2 * s : 2 * s + 2], cs[:, 5:7],
                    rbc2[:, s : s + 1], cs[:, 7:9], Alu.mult, Alu.add,
                )

            ndma = [0]

            def emit_block2(s):
                """Conv2 (24 MMs) + alternating drains + out DMAs, sample s."""
                for c in range(2):
                    ot = outp.tile([C1, WO], bf16, tag="ot")
                    sc_col = sc2[:, 2 * s + c : 2 * s + c + 1]
                    bi_col = bi2[:, 2 * s + c : 2 * s + c + 1]
                    for d in range(2):
                        zp2 = psum.tile([C1, W2T], f32, tag="zp")
                        for k in range(3):
                            for h in range(2):
                                t0 = W2T * d + WT * h
                                nc.tensor.matmul(
                                    zp2[:, WT * h : WT * h + WT],
                                    ws[:, W2OFF + k * C2 + C1 * c : W2OFF + k * C2 + C1 * c + C1],
                                    y1s[s][:, t0 + k : t0 + k + WT],
                                    start=(k == 0), stop=(k == 2),
                                )
                        if d == 0 or (s, c) in ((1, 0), (3, 0)):
                            # (a) VectorE drains PSUM: one-input 3D-AP
                            # max-pool -> bf16, then ScalarE Prelu(affine) at
                            # half width (exact: scale>0 keeps it monotone)
                            pm = pmp.tile([C1, WT], bf16, tag="pm")
                            nc.vector.tensor_reduce(
                                pm[:], zp2[:].rearrange("p (a b) -> p a b", b=2),
                                axis=Ax.X, op=Alu.max,
                            )
                            nc.scalar.activation(
                                ot[:, WT * d : WT * (d + 1)], pm[:], Act.Prelu,
                                bias=bi_col, scale=sc_col, alpha=SLOPE,
                            )
                        else:
                            # (b) ScalarE drains PSUM: full-width
                            # Prelu(affine) -> bf16, then VectorE pools
                            yw = pmp.tile([C1, W2T], bf16, tag="yw")
                            nc.scalar.activation(
                                yw[:], zp2[:], Act.Prelu,
                                bias=bi_col, scale=sc_col, alpha=SLOPE,
                            )
                            nc.vector.tensor_tensor(
                                ot[:, WT : 2 * WT],
                                yw[:, 0:W2T:2], yw[:, 1:W2T:2], Alu.max,
                            )
                        # ship each 512-col half as soon as it's final; the
                        # final tiles ride the fast SWDGE queue so the tail
                        # is not ring-bandwidth-bound
                        i = ndma[0]
                        ndma[0] += 1
                        eng = (nc.sync, nc.scalar)[i % 2] if i < 8 else nc.gpsimd
                        eng.dma_start(
                            out=o_ap[s, C1 * c : C1 * (c + 1), WT * d : WT * (d + 1)],
                            in_=ot[:, WT * d : WT * (d + 1)],
                        )

            # interleaved emission: PE queue never waits on routing chains,
            # every drain follows its scale/bias producer in program order,
            # and block-2 (with its out DMAs) runs 1 sample behind block-1
            zp0 = emit_b1_mms(0)
            warm_mms(3)
            emit_rt1(0)
            warm_mms(5)
            emit_b1_drains(0, zp0)
            zp1 = emit_b1_mms(1)
            emit_rt2(0)
            emit_rt1(1)
            emit_b1_drains(1, zp1)
            emit_block2(0)
            zp2 = emit_b1_mms(2)
            emit_rt2(1)
            emit_rt1(2)
            emit_b1_drains(2, zp2)
            emit_block2(1)
            zp3 = emit_b1_mms(3)
            emit_rt2(2)
            emit_rt1(3)
            emit_b1_drains(3, zp3)
            emit_block2(2)
            emit_rt2(3)
            emit_block2(3)

    nc.compile()
    _built = nc
    return nc


def _pack_inputs(x, w1, b1, fcw1, fcb1, g1, be1, rm1, rv1,
                 w2, b2, fcw2, fcb2, g2, be2, rm2, rv2):
    import ml_dtypes

    f = np.float32
    bf = np.dtype(ml_dtypes.bfloat16)
    s1 = (g1 / np.sqrt(rv1 + EPS)).astype(f)
    s2 = (g2 / np.sqrt(rv2 + EPS)).astype(f)
    t11, t21 = (b1[0] * s1).astype(f), (be1 - rm1 * s1).astype(f)
    t12, t22 = (b2[0] * s2).astype(f), (be2 - rm2 * s2).astype(f)

    # bf16 weights: w1 taps01 / tap2 lhsT, routing fc (replicated), w2 lhsT
    w1t = w1[0].transpose(1, 2, 0).astype(f)           # (CIN, 3, C1)
    w2t = w2[0].transpose(1, 2, 0).reshape(C1, 3 * C2).astype(f)
    wpk = np.zeros((C1, NWB), bf)
    wpk[0:CIN, 0:C1] = w1t[:, 0, :].astype(bf)
    wpk[CIN:C1, 0:C1] = w1t[:, 1, :].astype(bf)
    wpk[0:CIN, T2 : T2 + C1] = w1t[:, 2, :].astype(bf)
    wpk[:, FC1OFF : FC1OFF + C1] = (fcw1[0] / W)[:, None].astype(bf)[
        np.arange(C1) % CIN
    ]
    wpk[:, FC2OFF : FC2OFF + C1] = (fcw2[0] / W)[:, None].astype(bf)
    wpk[:, W2OFF:NWB] = w2t.astype(bf)

    # fp32 fused-affine consts
    cpk = np.zeros((C1, NCC), f)
    cols = [s1, t11, t21, s2[:C1], s2[C1:], t12[:C1], t12[C1:],
            t22[:C1], t22[C1:], np.full(C1, fcb1[0], f), np.full(C1, fcb2[0], f)]
    for j, col in enumerate(cols):
        cpk[:, j] = col

    # x duplicated into partitions 64..127 shifted left by one column, so
    # block-1 taps 0+1 contract in a single K=128 matmul
    xb = x.astype(bf)
    xpk = np.zeros((B, C1, W + 2), bf)
    xpk[:, 0:CIN, 1 : W + 1] = xb
    xpk[:, CIN:C1, 0:W] = xb

    com = {"wpk": wpk, "cpk": cpk}
    return [
        {**com, "xpk": np.ascontiguousarray(xpk[i * BL : (i + 1) * BL])}
        for i in range(N_CORES)
    ]


def _enable_trace():
    """Register the NTFF profile hook (absent antenv.axon_hooks on this image)
    and stub out the S3 artifact upload so trace=True works locally."""
    import sys
    import types

    import concourse.bass_utils as bu

    bu.upload_artifacts = lambda tmpdir: tmpdir
    if "antenv.axon_hooks" not in sys.modules:
        import antenv
        from trn_agent_boot.trn_boot import _ntff_profile_via_ctypes

        hooks = types.ModuleType("antenv.axon_hooks")
        _store = {"hook": _ntff_profile_via_ctypes("/opt/axon/libaxon_pjrt.so")}
        hooks.set_axon_ntff_profile_hook = lambda h: _store.__setitem__("hook", h)
        hooks.get_axon_ntff_profile_hook = lambda: _store["hook"]
        sys.modules["antenv.axon_hooks"] = hooks
        antenv.axon_hooks = hooks


def kernel(**inputs):
    global LAST_RESULT
    from concourse.bass_utils import run_bass_kernel_spmd

    if TRACE:
        _enable_trace()
    nc = _build()
    in_maps = _pack_inputs(**inputs)
    res = run_bass_kernel_spmd(nc, in_maps, list(range(N_CORES)), trace=TRACE)
    LAST_RESULT = res
    return np.concatenate(
        [r["out"].astype(np.float32) for r in res.results], axis=0
    )
